# revision 78
# baseline (speedup 1.0000x reference)
"""Trainium2 Bass kernel for a GQA attention layer (dense transformer).

Reference computation (B=1, S=2048, DIM=2048, 32 q-heads, 8 kv-heads, hd=64):
    xq = x @ wq; xk = x @ wk; xv = x @ wv
    rope(xq, xk); GQA causal attention; out = attn @ wo

Sharding: tensor-parallel over heads across 8 cores. Core c owns q-heads
4c..4c+3 (wq cols), kv-head c (wk/wv cols), and wo rows 256c..256c+255.
Each core computes a full [S, DIM] partial of the output projection; the
host sums the 8 partials (the TP all-reduce, done at gather time).

Kernel layout strategy (everything "transposed", head_dim on partitions):
  - QT/KT/VT = W.T @ x computed with lhsT = weight shard (natural [DIM, m]
    layout), rhs = x.T tiles streamed from DRAM.
  - RoPE pairs are de-interleaved by permuting wq/wk columns on the host so
    the rotation partner sits 16 partitions away (within a 32-partition
    quadrant, so DVE stream_shuffle can swap them).
  - Scores are computed transposed: S^T[k, q] = K^T.T @ Q^T per 128-row
    k-tile; exp on ACT (scale fused); causal mask = upper-tri multiply on
    the single diagonal 128x128 block of each k-tile.
  - P@V is computed as V'.T @ P^T where V' = [V | ones]: the ones column
    makes row 64 of the PSUM accumulator the softmax denominator for free.
  - Normalization: reciprocal (DVE) + partition_broadcast (GPSIMD) + mult.
  - Output projection from O^T with wo shard as rhs; partial DMA'd f32.
"""

import numpy as np
import ml_dtypes

import concourse.bass as bass
import concourse.mybir as mybir
from concourse import bacc
from concourse.tile import TileContext
from concourse.masks import make_identity
from concourse.bass_utils import run_bass_kernel_spmd

# ---------------------------------------------------------------- constants
S = 2048          # sequence length
DIM = 2048        # model dim
NH = 32           # query heads
NKV = 8           # kv heads
HD = 64           # head dim
NCORES = 8
HQ = NH // NCORES          # query heads per core = 4
QW = HQ * HD               # q width per core = 256
KT_S = S // 128            # 16 seq k-tiles
KT_D = DIM // 128          # 16 dim k-tiles
NSC = S // 512             # 4 s-chunks
SCALE = 1.0 / 8.0          # 1/sqrt(64)

# matmul dtype knob: 'bf16' | 'f32' | 'f32r'
MM = 'bf16'

_SHUF_SWAP16 = list(range(16, 32)) + list(range(16))


def _dtypes():
    if MM == 'bf16':
        return mybir.dt.bfloat16, mybir.dt.bfloat16, ml_dtypes.bfloat16
    if MM == 'f32':
        return mybir.dt.float32, mybir.dt.float32, np.float32
    if MM == 'f32r':
        return mybir.dt.float32, mybir.dt.float32r, np.float32
    raise ValueError(MM)


def _mm_ap(ap, mmdt):
    """View an AP in the matmul dtype (bitcast f32 -> f32r when needed)."""
    if ap.dtype != mmdt:
        return ap.bitcast(mmdt)
    return ap


def build_program():
    """Build the per-core Bass program (same program on all 8 cores).

    Emission is a fine-grained software pipeline: attention beats for chunk
    sc (S^T mega-matmul for head h + PV pairs of head h-1) are merged with
    the projection matmuls of chunk sc+1 and the WO units of chunk sc-1.
    Engines execute in order, so the merge keeps only likely-ready work in
    the PE stream while ACT (exp) and DVE (RoPE/normalize) drain.

    PSUM (8 banks): pjo 3 (projection passes + PV accumulators, shared tag)
    + ps 4 (two [128,1024] score megas) + pw 1 (WO).
    """
    sdt, mmdt, _ = _dtypes()
    f32 = mybir.dt.float32

    nc = bacc.Bacc("TRN2", target_bir_lowering=False, debug=False,
                   num_devices=NCORES)

    xT = nc.dram_tensor("xT", [DIM, S], sdt, kind="ExternalInput")
    wqkv = nc.dram_tensor("wqkv", [DIM, QW + 2 * HD], sdt,
                          kind="ExternalInput")
    wo_s = nc.dram_tensor("wo_s", [QW, DIM], sdt, kind="ExternalInput")
    cosE = nc.dram_tensor("cosE", [64, S], f32, kind="ExternalInput")
    sinE = nc.dram_tensor("sinE", [64, S], f32, kind="ExternalInput")
    utri = nc.dram_tensor("utri", [128, 128], sdt, kind="ExternalInput")
    out = nc.dram_tensor("out", [S, DIM], f32, kind="ExternalOutput")

    WQKV = QW + 2 * HD  # 384

    import contextlib
    with TileContext(nc) as tc, contextlib.ExitStack() as ctx:
        const = ctx.enter_context(tc.tile_pool(name="const", bufs=1))
        work = ctx.enter_context(tc.tile_pool(name="work", bufs=2))
        xtp = ctx.enter_context(tc.tile_pool(name="xtp", bufs=7))
        ptp = ctx.enter_context(tc.tile_pool(name="ptp", bufs=20))
        small = ctx.enter_context(tc.tile_pool(name="small", bufs=5))
        osb = ctx.enter_context(tc.tile_pool(name="osb", bufs=4))

        pjo = ctx.enter_context(tc.tile_pool(name="pjo", bufs=3,
                                             space="PSUM"))
        ps = ctx.enter_context(tc.tile_pool(name="ps", bufs=2, space="PSUM"))
        pw = ctx.enter_context(tc.tile_pool(name="pw", bufs=1, space="PSUM"))

        # ----------------------------------------------- persistent SBUF
        w_sb = const.tile([128, KT_D * WQKV], sdt, tag="w_sb")
        wo_sb = const.tile([128, 2 * DIM], sdt, tag="wo_sb")
        cos_sb = const.tile([128, S], f32, tag="cos_sb")
        sin_sb = const.tile([128, S], f32, tag="sin_sb")
        utri_sb = const.tile([128, 128], sdt, tag="utri_sb")
        ident = const.tile([128, 128], sdt, tag="ident")
        QT = const.tile([64, HQ * S], sdt, tag="QT")
        KVt = const.tile([128, S], sdt, tag="KVt")
        Vp = const.tile([128, KT_S * (HD + 1)], sdt, tag="Vp")
        OT = const.tile([128, 2 * S], sdt, tag="OT")

        make_identity(nc, ident[:])
        nc.gpsimd.memset(Vp[:], 1.0)  # ones columns for denominator

        wo_copy_flip = [0]

        # ---------------------------------------------- thunk generators
        def proj_thunks(sc, fused=False):
            """Projection of chunk sc: KV pass, K-rope, V transposes, then
            Q passes (one PSUM slot each, sequential). With fused=True
            (prologue) all three matmuls run per k-tile, using 3 slots."""
            s0 = sc * 512
            xts = []
            st = {}

            # uniform DMA batches of 4 k-tiles (HWDGE overhead per DMA
            # outweighs finer-grained arrival)
            batches = [4, 4, 4, 4]
            starts = [sum(batches[:i]) for i in range(len(batches))]
            kt_slot = {}
            for bi, (b0, bn) in enumerate(zip(starts, batches)):
                for j in range(bn):
                    kt_slot[b0 + j] = (bi, j, bn, b0)

            def dma_kv(kt):
                bi, j, bn, b0 = kt_slot[kt]
                if j == 0:
                    if sc == 0:
                        nc.sync.dma_start(
                            w_sb[:, b0 * WQKV:(b0 + bn) * WQKV].rearrange(
                                "r (k w) -> r k w", k=bn),
                            wqkv[b0 * 128:(b0 + bn) * 128, :].rearrange(
                                "(k r) w -> r k w", k=bn))
                    xt4 = xtp.tile([128, 4 * 512], sdt, tag="xt", name="xt4")
                    nc.sync.dma_start(
                        xt4[:, 0:bn * 512].rearrange("r (k c) -> r k c", k=bn),
                        xT[b0 * 128:(b0 + bn) * 128,
                           s0:s0 + 512].rearrange("(k r) c -> r k c", k=bn))
                    xts.append(xt4)
                    if sc == 0 and kt == 4:
                        # constants ride behind the first weight/x batches
                        # but land before the first RoPE needs them
                        # (cos/sin have 64-row periodicity: DMA'd twice)
                        nc.sync.dma_start(cos_sb[0:64, :], cosE[:])
                        nc.sync.dma_start(cos_sb[64:128, :], cosE[:])
                        nc.sync.dma_start(sin_sb[0:64, :], sinE[:])
                        nc.sync.dma_start(sin_sb[64:128, :], sinE[:])
                        nc.sync.dma_start(utri_sb[:], utri[:])
                bi, j, bn, b0 = kt_slot[kt]
                xt = xts[bi][:, j * 512:j * 512 + 512]
                if kt == 0:
                    st["pkv"] = pjo.tile([128, 512], f32, tag="pjo",
                                         name="pkv")
                    if fused:
                        st["fq0"] = pjo.tile([128, 512], f32, tag="pjo",
                                             name="fq0")
                        st["fq1"] = pjo.tile([128, 512], f32, tag="pjo",
                                             name="fq1")
                nc.tensor.matmul(
                    st["pkv"][:],
                    _mm_ap(w_sb[:, kt * WQKV + 256:kt * WQKV + 384], mmdt),
                    _mm_ap(xt, mmdt),
                    start=(kt == 0), stop=(kt == KT_D - 1))
                if fused:
                    for mt in range(2):
                        nc.tensor.matmul(
                            st[f"fq{mt}"][:],
                            _mm_ap(w_sb[:, kt * WQKV + mt * 128:
                                        kt * WQKV + mt * 128 + 128], mmdt),
                            _mm_ap(xt, mmdt),
                            start=(kt == 0), stop=(kt == KT_D - 1))


            def k_rope():
                pkv = st["pkv"]
                shufk = work.tile([64, 512], f32, tag="shufk", name="shufk")
                m1k = work.tile([64, 512], f32, tag="m1k", name="m1k")
                t2k = work.tile([64, 512], f32, tag="t2k", name="t2k")
                nc.vector.stream_shuffle(shufk[:], pkv[0:64, :],
                                         _SHUF_SWAP16)
                nc.vector.tensor_mul(m1k[:], pkv[0:64, :],
                                     cos_sb[0:64, s0:s0 + 512])
                nc.vector.tensor_mul(t2k[:], shufk[:],
                                     sin_sb[0:64, s0:s0 + 512])
                nc.vector.tensor_add(KVt[0:64, s0:s0 + 512], m1k[:], t2k[:])
                nc.vector.tensor_copy(KVt[64:128, s0:s0 + 512],
                                      pkv[64:128, :])

            def v_trans(kt):
                pv = pw.tile([128, HD], sdt, tag="pw", name="pv")
                nc.tensor.transpose(
                    pv[:], KVt[64:128, kt * 128:(kt + 1) * 128],
                    ident[64:128, 64:128])
                nc.vector.tensor_copy(
                    Vp[:, kt * (HD + 1):kt * (HD + 1) + HD], pv[:])

            def q_mm(mt, kt):
                if kt == 0:
                    st["pq"] = pjo.tile([128, 512], f32, tag="pjo",
                                        name="pq")
                w0 = kt * WQKV + mt * 128
                bi, j, bn, b0 = kt_slot[kt]
                xt = xts[bi][:, j * 512:j * 512 + 512]
                nc.tensor.matmul(
                    st["pq"][:], _mm_ap(w_sb[:, w0:w0 + 128], mmdt),
                    _mm_ap(xt, mmdt),
                    start=(kt == 0), stop=(kt == KT_D - 1))

            def q_rope(mt):
                pq = st[f"fq{mt}"] if fused else st["pq"]
                shuf = work.tile([128, 512], f32, tag="shuf", name="shuf")
                m1 = work.tile([128, 512], f32, tag="m1", name="m1")
                t2 = work.tile([128, 512], f32, tag="t2", name="t2")
                nc.vector.stream_shuffle(shuf[:], pq[:], _SHUF_SWAP16)
                nc.vector.tensor_mul(m1[:], pq[:], cos_sb[:, s0:s0 + 512])
                nc.vector.tensor_mul(t2[:], shuf[:], sin_sb[:, s0:s0 + 512])
                he = (2 * mt) * S
                ho = (2 * mt + 1) * S
                nc.vector.tensor_add(
                    QT[:, he + s0:he + s0 + 512], m1[0:64, :], t2[0:64, :])
                nc.vector.tensor_add(
                    QT[:, ho + s0:ho + s0 + 512], m1[64:128, :],
                    t2[64:128, :])

            th = [lambda kt=kt: dma_kv(kt) for kt in range(KT_D)]
            th.append(k_rope)
            th += [lambda kt=kt: v_trans(kt)
                   for kt in range(4 * sc, 4 * sc + 4)]
            if fused:
                th += [lambda mt=mt: q_rope(mt) for mt in range(2)]
            else:
                for mt in range(2):
                    th += [lambda mt=mt, kt=kt: q_mm(mt, kt)
                           for kt in range(KT_D)]
                    th.append(lambda mt=mt: q_rope(mt))
            return th

        def s_thunks(qc, h, tiles):
            """S^T mega matmuls + exp + mask for one head; fills `tiles`."""
            q0 = qc * 512
            hf = h * S
            nkt = 4 * qc + 4
            thunks = []
            for pi in range(nkt // 2):
                def th(pi=pi):
                    kts = (2 * pi, 2 * pi + 1)
                    ps_t = ps.tile([128, 1024], f32, tag="ps", name="ps_t")
                    pt_t = ptp.tile([128, 1024], sdt, tag="pt", name="pt_t")
                    for li, kt in enumerate(kts):
                        dj = kt - 4 * qc
                        qo = 128 * dj if dj >= 0 else 0
                        lo = li * 512
                        nc.tensor.matmul(
                            ps_t[:, lo + qo:lo + 512],
                            _mm_ap(KVt[0:64, kt * 128:(kt + 1) * 128], mmdt),
                            _mm_ap(QT[:, hf + q0 + qo:hf + q0 + 512], mmdt),
                            start=True, stop=True)
                    if 2 * pi + 1 < 4 * qc:
                        nc.scalar.activation(
                            pt_t[:], ps_t[:],
                            mybir.ActivationFunctionType.Exp, scale=SCALE)
                    else:
                        for li, kt in enumerate(kts):
                            dj = kt - 4 * qc
                            qo = 128 * dj if dj >= 0 else 0
                            lo = li * 512
                            nc.scalar.activation(
                                pt_t[:, lo + qo:lo + 512],
                                ps_t[:, lo + qo:lo + 512],
                                mybir.ActivationFunctionType.Exp,
                                scale=SCALE)
                    for li, kt in enumerate(kts):
                        dj = kt - 4 * qc
                        qo = 128 * dj if dj >= 0 else 0
                        lo = li * 512
                        if dj >= 0:
                            nc.vector.tensor_mul(
                                pt_t[:, lo + qo:lo + qo + 128],
                                pt_t[:, lo + qo:lo + qo + 128], utri_sb[:])
                        tiles.append((kt, qo, lo, pt_t))
                thunks.append(th)
            return thunks

        def pv_thunks(qc, h, tiles):
            """PV accumulation pairs + final normalization for one head."""
            q0 = qc * 512
            hp = (h % 2) * 64
            nkt0 = 4 * qc + 4
            state = {}

            def pv_pair(pi):
                if "po" not in state:
                    state["po"] = pjo.tile([HD + 1, 512], f32, tag="pjo",
                                           name="pot")
                po_t = state["po"]
                for kt, qo, lo, pt_t in tiles[2 * pi:2 * pi + 2]:
                    nc.tensor.matmul(
                        po_t[:, qo:512],
                        _mm_ap(Vp[:, kt * (HD + 1):(kt + 1) * (HD + 1)],
                               mmdt),
                        _mm_ap(pt_t[:, lo + qo:lo + 512], mmdt),
                        start=(kt == 0), stop=(kt == nkt0 - 1))
                if 2 * pi + 2 >= nkt0:
                    rc = small.tile([1, 512], f32, tag="rc", name="rc")
                    rb = small.tile([64, 512], f32, tag="rb", name="rb")
                    nc.vector.reciprocal(rc[:], po_t[64:65, :])
                    nc.gpsimd.partition_broadcast(rb[:], rc[:])
                    of = (h // 2) * S
                    nc.vector.tensor_mul(
                        OT[hp:hp + 64, of + q0:of + q0 + 512],
                        po_t[0:64, :], rb[:])

            return [lambda pi=pi: pv_pair(pi) for pi in range(nkt0 // 2)]

        def wo_half(qt, np2, half, obs, pool=None, ptag="pw",
                    act_copy=False, split_dma=False):
            """One 512-wide n-chunk; the second half fires the paired
            [128,1024] output DMA (or each half its own when split_dma,
            used in the epilogue to shorten the tail)."""
            pool = pool or pw
            if half == 0:
                obs[(qt, np2)] = osb.tile([128, 1024], f32, tag="ob",
                                          name="ob")
            ob = obs[(qt, np2)]
            ncn = 2 * np2 + half
            pw_t = pool.tile([128, 512], f32, tag=ptag, name="pw_t")
            for mt in range(2):
                nc.tensor.matmul(
                    pw_t[:],
                    _mm_ap(OT[:, mt * S + qt * 128:
                              mt * S + (qt + 1) * 128], mmdt),
                    _mm_ap(wo_sb[:, mt * DIM + ncn * 512:
                                 mt * DIM + ncn * 512 + 512], mmdt),
                    start=(mt == 0), stop=(mt == 1))
            if act_copy:
                nc.scalar.copy(ob[:, half * 512:half * 512 + 512], pw_t[:])
            else:
                nc.vector.tensor_copy(
                    ob[:, half * 512:half * 512 + 512], pw_t[:])
            if split_dma:
                nc.sync.dma_start(
                    out[qt * 128:(qt + 1) * 128, ncn * 512:ncn * 512 + 512],
                    ob[:, half * 512:half * 512 + 512])
                if half == 1:
                    del obs[(qt, np2)]
                    wo_copy_flip[0] += 1
            elif half == 1:
                del obs[(qt, np2)]
                wo_copy_flip[0] += 1
                nc.sync.dma_start(
                    out[qt * 128:(qt + 1) * 128,
                        np2 * 1024:np2 * 1024 + 1024], ob[:])
        wo_obs = {}

        # ------------------------------------- merged emission schedule
        def merge(primary, *others):
            """Emit primary thunks; proportionally interleave the others."""
            counters = [0.0] * len(others)
            n = max(1, len(primary))
            for beat in primary:
                for j, lst in enumerate(others):
                    counters[j] += len(lst) / n
                    while counters[j] >= 1.0 and lst:
                        lst.pop(0)()
                        counters[j] -= 1.0
                for th in beat:
                    th()
            for lst in others:
                while lst:
                    lst.pop(0)()

        for th in proj_thunks(0, fused=True):       # prologue
            th()

        prev = None                      # (qc, h, tiles) awaiting PV
        for sc in range(NSC):
            if sc == 1:
                nc.sync.dma_start(wo_sb[:, 0:DIM], wo_s[0:128, :])
                nc.sync.dma_start(wo_sb[:, DIM:2 * DIM], wo_s[128:256, :])
            pstream = proj_thunks(sc + 1) if sc + 1 < NSC else []
            wostream = ([lambda qt=qt, np2=np2, half=half:
                         wo_half(qt, np2, half, wo_obs)
                         for qt in range(4 * (sc - 1), 4 * (sc - 1) + 4)
                         for np2 in range(2)
                         for half in range(2)] if sc >= 1 else [])
            for h in range(HQ):
                tiles = []
                sth = s_thunks(sc, h, tiles)
                pth = pv_thunks(*prev) if prev is not None else []
                beats = []
                for i in range(max(len(sth), len(pth))):
                    beat = []
                    if i < len(pth):
                        beat.append(pth[i])
                    if i < len(sth):
                        beat.append(sth[i])
                    beats.append(beat)
                # WO of sc-1 needs PV(sc-1, 3) done: that PV is head 0 here
                if h == 0:
                    ptake = max(1, len(pstream) // HQ) if pstream else 0
                    merge(beats, pstream[:ptake])
                    pstream = pstream[ptake:]
                else:
                    ptake = (len(pstream) // (HQ - h)) if pstream else 0
                    wtake = (len(wostream) // (HQ - h)) if wostream else 0
                    merge(beats, pstream[:ptake], wostream[:wtake])
                    pstream = pstream[ptake:]
                    wostream = wostream[wtake:]
                prev = (sc, h, tiles)
            merge([], pstream, wostream)

        # epilogue: PV of the last head, then WO of chunk 3; the score
        # pool's banks are free now, so WO rotates through those too
        for th in pv_thunks(*prev):
            th()
        epi = 0
        pools = [(pw, "pw"), (ps, "ps"), (pjo, "pjo")]
        for qt in range(12, 16):
            for np2 in range(2):
                pool, ptag = pools[epi % 3]
                for half in range(2):
                    wo_half(qt, np2, half, wo_obs, pool=pool, ptag=ptag,
                            act_copy=(epi % 2 == 0), split_dma=True)
                epi += 1

    nc.compile()
    return nc


# ------------------------------------------------------------- host side
def _pair_perm64():
    """Column permutation putting the RoPE partner 16 partitions away."""
    return np.array([2 * (16 * (j // 32) + (j % 16)) + ((j % 32) // 16)
                     for j in range(64)])


def _host_prep(x, freqs_cos, freqs_sin, wq, wk, wv, wo):
    _, _, npdt = _dtypes()
    x = np.asarray(x, np.float32)
    fc = np.asarray(freqs_cos, np.float32)
    fs = np.asarray(freqs_sin, np.float32)
    wq = np.asarray(wq, np.float32)
    wk = np.asarray(wk, np.float32)
    wv = np.asarray(wv, np.float32)
    wo = np.asarray(wo, np.float32)

    perm = _pair_perm64()
    xT = np.ascontiguousarray(x[0].T).astype(npdt)

    p = np.arange(64)
    pair = 16 * ((p % 64) // 32) + (p % 16)
    sign = np.where((p % 32) < 16, -1.0, 1.0).astype(np.float32)
    cosE = np.ascontiguousarray(fc[:, pair].T)                  # [64, S]
    sinE = np.ascontiguousarray(fs[:, pair].T) * sign[:, None]  # [64, S]
    utri = np.triu(np.ones((128, 128), np.float32)).astype(npdt)

    in_maps = []
    for c in range(NCORES):
        qcols = np.concatenate(
            [wq[:, (4 * c + i) * 64 + perm] for i in range(HQ)], axis=1)
        kcols = wk[:, c * 64 + perm]
        vcols = wv[:, c * 64:(c + 1) * 64]
        wqkv_c = np.concatenate([qcols, kcols, vcols], axis=1).astype(npdt)
        wo_c = wo[QW * c:QW * (c + 1), :].astype(npdt)
        in_maps.append({
            "xT": xT,
            "wqkv": np.ascontiguousarray(wqkv_c),
            "wo_s": np.ascontiguousarray(wo_c),
            "cosE": cosE.astype(np.float32),
            "sinE": np.ascontiguousarray(sinE).astype(np.float32),
            "utri": np.ascontiguousarray(utri),
        })
    return in_maps


_NC_CACHE = {}


def get_program():
    if MM not in _NC_CACHE:
        _NC_CACHE[MM] = build_program()
    return _NC_CACHE[MM]


def kernel(x, freqs_cos, freqs_sin, wq, wk, wv, wo):
    nc = get_program()
    in_maps = _host_prep(x, freqs_cos, freqs_sin, wq, wk, wv, wo)
    res = run_bass_kernel_spmd(nc, in_maps, core_ids=list(range(NCORES)))
    acc = np.zeros((S, DIM), np.float64)
    for r in res.results:
        acc += r["out"].astype(np.float64)
    return acc.astype(np.float32).reshape(1, S, DIM)
